# revision 11
# baseline (speedup 1.0000x reference)
"""Trainium2 Bass kernel for the Non-local block (rank-1 collapsed form).

Math (per batch b, with xf = x.reshape(B, C, N)):
    g    = g_w . xf + g_b              (B, N)
    phi  = phi_w . xf + phi_b          (B, N)
    s    = sum(phi * g, n) / N         (B,)
    theta= theta_w . xf + theta_b      (B, N)
    z    = x + A * s * theta + D       A = W_w*inv_std, D = (W_b-mean)*inv_std+beta

which collapses to one 256x256 matmul per batch plus a bias:
    W3[k, c] = I[k, c] + s_b * theta_w[k] * A[c] / N'   (N' folded into s)
    E[c]     = s_b * theta_b * A[c] + D[c]
    z[b]     = W3(s_b)^T @ x[b] + E

Per-core schedule (data-parallel over batch, 4 batches/core):
  phase 1: PE computes u=(g+phi), v=(g-phi) rows (M=2 matmul, biases via a
           K=1 ones-matmul); ACT squares PSUM->SBUF; DVE scales by +-0.25/N
           with per-partition accumulate (s = sum(u^2-v^2)/(4N)).
  s-chain: GPSIMD all-reduces 2 partitions + broadcasts s to 128 partitions;
           DVE builds W3 and E.
  phase 2: PE computes W3^T @ x into PSUM, ACT adds per-channel bias E while
           copying PSUM->SBUF, DMA stores z.
"""

import sys

sys.path.insert(0, "/opt/trn_rl_repo")

import numpy as np

B, C, HH, WW = 32, 256, 96, 48
N = HH * WW  # 4608
P = 128
NCORES = 8
BLOC = B // NCORES  # 4
FD = 512
NCH = N // FD  # 9
BN_EPS = 1e-5

_cache = {}


def _build_nc():
    from contextlib import ExitStack

    import concourse.tile as tile
    from concourse import bacc, mybir
    from concourse import bass_isa

    f32 = mybir.dt.float32
    f32r = mybir.dt.float32r
    mult = mybir.AluOpType.mult
    addop = mybir.AluOpType.add
    Copy = mybir.ActivationFunctionType.Copy
    Square = mybir.ActivationFunctionType.Square
    Ident = mybir.ActivationFunctionType.Identity

    nc = bacc.Bacc("TRN2", target_bir_lowering=False, debug=False)

    x_d = nc.dram_tensor("x", [BLOC, 2, P, N], f32r, kind="ExternalInput").ap()
    z_d = nc.dram_tensor("z", [BLOC, 2, P, N], f32, kind="ExternalOutput").ap()
    # consts: wgp [P,4] = [g_w|phi_w] per k-block; bgp [1,2] = [g_b, phi_b]
    # w2b [P,512] = theta_w[k-block] x A / N per k-block; i2 = identity blocks
    # fd [P,4] = [F0, F1, D0, D1] with F = theta_b * A
    wgp_d = nc.dram_tensor("wgp", [P, 4], f32r, kind="ExternalInput").ap()
    bgp_d = nc.dram_tensor("bgp", [2, 1], f32, kind="ExternalInput").ap()
    w2b_d = nc.dram_tensor("w2b", [P, 512], f32, kind="ExternalInput").ap()
    i2_d = nc.dram_tensor("i2", [P, 512], f32, kind="ExternalInput").ap()
    fd_d = nc.dram_tensor("fd", [P, 4], f32, kind="ExternalInput").ap()
    pm_d = nc.dram_tensor("pm", [2, 1], f32, kind="ExternalInput").ap()

    with tile.TileContext(nc) as tc, ExitStack() as ctx:
        const = ctx.enter_context(tc.tile_pool(name="const", bufs=1))
        xpool = ctx.enter_context(tc.tile_pool(name="xpool", bufs=4))
        zpool = ctx.enter_context(tc.tile_pool(name="zpool", bufs=6))
        wpool = ctx.enter_context(tc.tile_pool(name="wpool", bufs=2))
        spool = ctx.enter_context(tc.tile_pool(name="spool", bufs=3))
        gpool = ctx.enter_context(tc.tile_pool(name="gpool", bufs=4))
        p1pool = ctx.enter_context(tc.tile_pool(name="p1pool", bufs=2, space="PSUM"))
        p2pool = ctx.enter_context(tc.tile_pool(name="p2pool", bufs=2, space="PSUM"))

        wgp = const.tile([P, 4], f32r)
        nc.sync.dma_start(wgp[:], wgp_d[:])
        bgp = const.tile([2, 1], f32)
        nc.sync.dma_start(bgp[:], bgp_d[:])
        w2b = const.tile([P, 512], f32)
        nc.sync.dma_start(w2b[:], w2b_d[:])
        i2 = const.tile([P, 512], f32)
        nc.sync.dma_start(i2[:], i2_d[:])
        fdc = const.tile([P, 4], f32)
        nc.sync.dma_start(fdc[:], fd_d[:])
        pm = const.tile([2, 1], f32)
        nc.sync.dma_start(pm[:], pm_d[:])

        xts = {}
        accrs = {}

        def load_ph1(b):
            xt0 = xpool.tile([P, N], f32r, name=f"xt0_b{b}", tag="xt0")
            xt1 = xpool.tile([P, N], f32r, name=f"xt1_b{b}", tag="xt1")
            nc.sync.dma_start(xt0[:], x_d[b, 0])
            nc.sync.dma_start(xt1[:], x_d[b, 1])
            xts[b] = (xt0, xt1)
            accr = spool.tile([2, NCH], f32, name=f"accr_b{b}", tag="accr")
            for j in range(NCH):
                js = slice(j * FD, (j + 1) * FD)
                ps1 = p1pool.tile([2, FD], f32, name=f"ps1_b{b}_{j}", tag="ps1")
                nc.tensor.matmul(
                    ps1[:], wgp[:, 0:2], xt0[:, js],
                    start=True, stop=False,
                )
                nc.tensor.matmul(
                    ps1[:], wgp[:, 2:4], xt1[:, js],
                    start=False, stop=True,
                )
                sq = gpool.tile([2, FD], f32, name=f"sq_b{b}_{j}", tag="sq")
                nc.scalar.activation(
                    sq[:], ps1[:], Square,
                    bias=bgp[:, 0:1], scale=1.0,
                    accum_out=accr[:, j : j + 1],
                )
            accrs[b] = accr

        def schain(b):
            scr2 = spool.tile([2, NCH], f32, name=f"scr2_b{b}", tag="scr2")
            acc2 = spool.tile([2, 1], f32, name=f"acc2_b{b}", tag="acc2")
            nc.vector.tensor_scalar(
                out=scr2[:],
                in0=accrs[b][:],
                scalar1=pm[:, 0:1],
                scalar2=0.0,
                op0=mult,
                op1=addop,
                accum_out=acc2[:],
            )
            acc2r = spool.tile([2, 1], f32, name=f"acc2r_b{b}", tag="acc2r")
            nc.gpsimd.partition_all_reduce(
                acc2r[:], acc2[:], channels=2, reduce_op=bass_isa.ReduceOp.add
            )
            s_sb = spool.tile([P, 1], f32, name=f"ssb_b{b}", tag="ssb")
            nc.gpsimd.partition_broadcast(s_sb[:], acc2r[0:1, 0:1])
            ecol = spool.tile([P, 2], f32, name=f"ecol_b{b}", tag="ecol")
            for cb in range(2):
                nc.vector.tensor_scalar(
                    out=ecol[:, cb : cb + 1],
                    in0=fdc[:, cb : cb + 1],
                    scalar1=s_sb[:, 0:1],
                    scalar2=fdc[:, 2 + cb : 3 + cb],
                    op0=mult,
                    op1=addop,
                )
            w3 = wpool.tile([P, 512], f32r, name=f"w3_b{b}", tag="w3")
            for kb in range(2):
                ks = slice(kb * 256, (kb + 1) * 256)
                nc.vector.scalar_tensor_tensor(
                    out=w3[:, ks],
                    in0=w2b[:, ks],
                    scalar=s_sb[:, 0:1],
                    in1=i2[:, ks],
                    op0=mult,
                    op1=addop,
                )
            return ecol, w3

        JGS = [(0, 3), (3, 6), (6, 9)]

        def ph2_store(b, ecol, w3):
            xt = xts[b]
            for cb in range(2):
                for gi, (j0, j1) in enumerate(JGS):
                    gw = (j1 - j0) * FD
                    ps2 = p2pool.tile(
                        [P, gw], f32, name=f"ps2_b{b}_{cb}_{j0}", tag="ps2"
                    )
                    for kb in range(2):
                        lhsT = w3[:, kb * 256 + cb * P : kb * 256 + (cb + 1) * P]
                        for j in range(j0, j1):
                            nc.tensor.matmul(
                                ps2[:, (j - j0) * FD : (j - j0 + 1) * FD],
                                lhsT,
                                xt[kb][:, j * FD : (j + 1) * FD],
                                start=(kb == 0),
                                stop=(kb == 1),
                            )
                    zt = zpool.tile([P, 3 * FD], f32, name=f"zt_b{b}_{cb}_{j0}", tag="zt")
                    if (gi + cb) % 2 == 0:
                        nc.scalar.activation(
                            zt[:, :gw],
                            ps2[:],
                            Ident,
                            bias=ecol[:, cb : cb + 1],
                            scale=1.0,
                        )
                    else:
                        nc.vector.tensor_scalar(
                            out=zt[:, :gw],
                            in0=ps2[:],
                            scalar1=ecol[:, cb : cb + 1],
                            scalar2=None,
                            op0=addop,
                        )
                    nc.sync.dma_start(
                        z_d[b, cb, :, j0 * FD : j1 * FD], zt[:, :gw]
                    )

        load_ph1(0)
        load_ph1(1)
        for b in range(BLOC):
            if b + 2 < BLOC:
                load_ph1(b + 2)
            ecol, w3 = schain(b)
            ph2_store(b, ecol, w3)

    nc.compile()
    return nc


def _get_nc():
    if "nc" not in _cache:
        _cache["nc"] = _build_nc()
    return _cache["nc"]


def kernel(x, g_w, g_b, theta_w, theta_b, phi_w, phi_b, W_w, W_b,
           bn_gamma, bn_beta, bn_mean, bn_var):
    import os

    from concourse.bass_utils import run_bass_kernel_spmd

    x = np.ascontiguousarray(np.asarray(x, dtype=np.float32))
    f = lambda a: np.asarray(a, dtype=np.float32).reshape(-1)
    g_w, g_b = f(g_w), f(g_b)
    theta_w, theta_b = f(theta_w), f(theta_b)
    phi_w, phi_b = f(phi_w), f(phi_b)
    W_w, W_b = f(W_w), f(W_b)
    bn_gamma, bn_beta = f(bn_gamma), f(bn_beta)
    bn_mean, bn_var = f(bn_mean), f(bn_var)

    inv_std = bn_gamma / np.sqrt(bn_var + BN_EPS)
    A = W_w * inv_std                       # (C,)
    D = (W_b - bn_mean) * inv_std + bn_beta  # (C,)

    w_u = g_w + phi_w
    w_v = g_w - phi_w
    wgp = np.empty((P, 4), np.float32)
    wgp[:, 0] = w_u[:P]
    wgp[:, 1] = w_v[:P]
    wgp[:, 2] = w_u[P:]
    wgp[:, 3] = w_v[P:]
    bgp = np.array([[g_b[0] + phi_b[0]], [g_b[0] - phi_b[0]]], np.float32)
    pm = np.array([[0.25 / N], [-0.25 / N]], np.float32)
    # w2b[k, kb*256 + c] = theta_w[kb*128 + k] * A[c]
    w2b = np.empty((P, 512), np.float32)
    w2b[:, 0:256] = theta_w[:P, None] * A[None, :]
    w2b[:, 256:512] = theta_w[P:, None] * A[None, :]
    i2 = np.zeros((P, 512), np.float32)
    i2[np.arange(P), np.arange(P)] = 1.0          # kb=0 block: rows k == cols c
    i2[np.arange(P), 256 + P + np.arange(P)] = 1.0  # kb=1: k+128 == c
    fd = np.empty((P, 4), np.float32)
    F = theta_b[0] * A
    fd[:, 0] = F[:P]
    fd[:, 1] = F[P:]
    fd[:, 2] = D[:P]
    fd[:, 3] = D[P:]

    consts = {
        "wgp": wgp, "bgp": bgp,
        "w2b": w2b, "i2": i2, "fd": fd, "pm": pm,
    }

    in_maps = []
    for i in range(NCORES):
        xs = np.ascontiguousarray(
            x[i * BLOC : (i + 1) * BLOC].reshape(BLOC, 2, P, N)
        )
        in_maps.append({"x": xs, **consts})

    nc = _get_nc()
    trace = os.environ.get("KERNEL_TRACE", "0") == "1"
    res = run_bass_kernel_spmd(
        nc, in_maps, core_ids=list(range(NCORES)), trace=trace
    )
    _cache["last_results"] = res

    z = np.concatenate(
        [res.results[i]["z"].reshape(BLOC, C, HH, WW) for i in range(NCORES)],
        axis=0,
    )
    return z


# revision 12
# speedup vs baseline: 1.1679x; 1.1679x over previous
"""Trainium2 Bass kernel for the Non-local block (rank-1 collapsed form).

Math (per batch b, with xf = x.reshape(B, C, N)):
    g    = g_w . xf + g_b              (B, N)
    phi  = phi_w . xf + phi_b          (B, N)
    s    = sum(phi * g, n) / N         (B,)
    theta= theta_w . xf + theta_b      (B, N)
    z    = x + A * s * theta + D       A = W_w*inv_std, D = (W_b-mean)*inv_std+beta

which collapses to one 256x256 matmul per batch plus a bias:
    W3[k, c] = I[k, c] + s_b * theta_w[k] * A[c] / N'   (N' folded into s)
    E[c]     = s_b * theta_b * A[c] + D[c]
    z[b]     = W3(s_b)^T @ x[b] + E

Per-core schedule (data-parallel over batch, 4 batches/core):
  phase 1: PE computes u=(g+phi), v=(g-phi) rows (M=2 matmul, biases via a
           K=1 ones-matmul); ACT squares PSUM->SBUF; DVE scales by +-0.25/N
           with per-partition accumulate (s = sum(u^2-v^2)/(4N)).
  s-chain: GPSIMD all-reduces 2 partitions + broadcasts s to 128 partitions;
           DVE builds W3 and E.
  phase 2: PE computes W3^T @ x into PSUM, ACT adds per-channel bias E while
           copying PSUM->SBUF, DMA stores z.
"""

import sys

sys.path.insert(0, "/opt/trn_rl_repo")

import numpy as np

B, C, HH, WW = 32, 256, 96, 48
N = HH * WW  # 4608
P = 128
NCORES = 8
BLOC = B // NCORES  # 4
FD = 512
NCH = N // FD  # 9
BN_EPS = 1e-5

_cache = {}


def _build_nc():
    from contextlib import ExitStack

    import concourse.tile as tile
    from concourse import bacc, mybir
    from concourse import bass_isa

    f32 = mybir.dt.float32
    f32r = mybir.dt.float32r
    mult = mybir.AluOpType.mult
    addop = mybir.AluOpType.add
    Copy = mybir.ActivationFunctionType.Copy
    Square = mybir.ActivationFunctionType.Square
    Ident = mybir.ActivationFunctionType.Identity

    nc = bacc.Bacc("TRN2", target_bir_lowering=False, debug=False)

    x_d = nc.dram_tensor("x", [BLOC, 2, P, N], f32r, kind="ExternalInput").ap()
    z_d = nc.dram_tensor("z", [BLOC, 2, P, N], f32, kind="ExternalOutput").ap()
    # consts: wgp [P,4] = [g_w|phi_w] per k-block; bgp [1,2] = [g_b, phi_b]
    # w2b [P,512] = theta_w[k-block] x A / N per k-block; i2 = identity blocks
    # fd [P,4] = [F0, F1, D0, D1] with F = theta_b * A
    wgp_d = nc.dram_tensor("wgp", [P, 4], f32r, kind="ExternalInput").ap()
    bgp_d = nc.dram_tensor("bgp", [2, 1], f32, kind="ExternalInput").ap()
    w2b_d = nc.dram_tensor("w2b", [P, 512], f32, kind="ExternalInput").ap()
    i2_d = nc.dram_tensor("i2", [P, 512], f32, kind="ExternalInput").ap()
    fd_d = nc.dram_tensor("fd", [P, 4], f32, kind="ExternalInput").ap()
    pm_d = nc.dram_tensor("pm", [2, 1], f32, kind="ExternalInput").ap()

    with tile.TileContext(nc) as tc, ExitStack() as ctx:
        const = ctx.enter_context(tc.tile_pool(name="const", bufs=1))
        xpool = ctx.enter_context(tc.tile_pool(name="xpool", bufs=4))
        zpool = ctx.enter_context(tc.tile_pool(name="zpool", bufs=6))
        wpool = ctx.enter_context(tc.tile_pool(name="wpool", bufs=2))
        spool = ctx.enter_context(tc.tile_pool(name="spool", bufs=3))
        gpool = ctx.enter_context(tc.tile_pool(name="gpool", bufs=4))
        p1pool = ctx.enter_context(tc.tile_pool(name="p1pool", bufs=2, space="PSUM"))
        p2pool = ctx.enter_context(tc.tile_pool(name="p2pool", bufs=2, space="PSUM"))

        wgp = const.tile([P, 4], f32r)
        nc.sync.dma_start(wgp[:], wgp_d[:])
        bgp = const.tile([2, 1], f32)
        nc.sync.dma_start(bgp[:], bgp_d[:])
        w2b = const.tile([P, 512], f32)
        nc.sync.dma_start(w2b[:], w2b_d[:])
        i2 = const.tile([P, 512], f32)
        nc.sync.dma_start(i2[:], i2_d[:])
        fdc = const.tile([P, 4], f32)
        nc.sync.dma_start(fdc[:], fd_d[:])
        pm = const.tile([2, 1], f32)
        nc.sync.dma_start(pm[:], pm_d[:])

        xts = {}
        accrs = {}

        def load_ph1(b):
            xt0 = xpool.tile([P, N], f32r, name=f"xt0_b{b}", tag="xt0")
            xt1 = xpool.tile([P, N], f32r, name=f"xt1_b{b}", tag="xt1")
            nc.sync.dma_start(xt0[:], x_d[b, 0])
            nc.sync.dma_start(xt1[:], x_d[b, 1])
            xts[b] = (xt0, xt1)
            accr = spool.tile([2, NCH], f32, name=f"accr_b{b}", tag="accr")
            for j in range(NCH):
                js = slice(j * FD, (j + 1) * FD)
                ps1 = p1pool.tile([2, FD], f32, name=f"ps1_b{b}_{j}", tag="ps1")
                nc.tensor.matmul(
                    ps1[:], wgp[:, 0:2], xt0[:, js],
                    start=True, stop=False,
                )
                nc.tensor.matmul(
                    ps1[:], wgp[:, 2:4], xt1[:, js],
                    start=False, stop=True,
                )
                sq = gpool.tile([2, FD], f32, name=f"sq_b{b}_{j}", tag="sq")
                nc.scalar.activation(
                    sq[:], ps1[:], Square,
                    bias=bgp[:, 0:1], scale=1.0,
                    accum_out=accr[:, j : j + 1],
                )
            accrs[b] = accr

        def schain(b):
            scr2 = spool.tile([2, NCH], f32, name=f"scr2_b{b}", tag="scr2")
            acc2 = spool.tile([2, 1], f32, name=f"acc2_b{b}", tag="acc2")
            nc.vector.tensor_scalar(
                out=scr2[:],
                in0=accrs[b][:],
                scalar1=pm[:, 0:1],
                scalar2=0.0,
                op0=mult,
                op1=addop,
                accum_out=acc2[:],
            )
            acc2r = spool.tile([2, 1], f32, name=f"acc2r_b{b}", tag="acc2r")
            nc.gpsimd.partition_all_reduce(
                acc2r[:], acc2[:], channels=2, reduce_op=bass_isa.ReduceOp.add
            )
            s_sb = spool.tile([P, 1], f32, name=f"ssb_b{b}", tag="ssb")
            nc.gpsimd.partition_broadcast(s_sb[:], acc2r[0:1, 0:1])
            ecol = spool.tile([P, 2], f32, name=f"ecol_b{b}", tag="ecol")
            for cb in range(2):
                nc.vector.tensor_scalar(
                    out=ecol[:, cb : cb + 1],
                    in0=fdc[:, cb : cb + 1],
                    scalar1=s_sb[:, 0:1],
                    scalar2=fdc[:, 2 + cb : 3 + cb],
                    op0=mult,
                    op1=addop,
                )
            w3 = wpool.tile([P, 512], f32r, name=f"w3_b{b}", tag="w3")
            for kb in range(2):
                ks = slice(kb * 256, (kb + 1) * 256)
                nc.vector.scalar_tensor_tensor(
                    out=w3[:, ks],
                    in0=w2b[:, ks],
                    scalar=s_sb[:, 0:1],
                    in1=i2[:, ks],
                    op0=mult,
                    op1=addop,
                )
            return ecol, w3

        JGS = [(0, 3), (3, 6), (6, 9)]

        def ph2_store(b, ecol, w3):
            xt = xts[b]
            for cb in range(2):
                for gi, (j0, j1) in enumerate(JGS):
                    gw = (j1 - j0) * FD
                    ps2 = p2pool.tile(
                        [P, gw], f32, name=f"ps2_b{b}_{cb}_{j0}", tag="ps2"
                    )
                    for kb in range(2):
                        lhsT = w3[:, kb * 256 + cb * P : kb * 256 + (cb + 1) * P]
                        for j in range(j0, j1):
                            nc.tensor.matmul(
                                ps2[:, (j - j0) * FD : (j - j0 + 1) * FD],
                                lhsT,
                                xt[kb][:, j * FD : (j + 1) * FD],
                                start=(kb == 0),
                                stop=(kb == 1),
                            )
                    zt = zpool.tile([P, 3 * FD], f32, name=f"zt_b{b}_{cb}_{j0}", tag="zt")
                    if (gi + cb) % 2 == 0:
                        nc.scalar.activation(
                            zt[:, :gw],
                            ps2[:],
                            Ident,
                            bias=ecol[:, cb : cb + 1],
                            scale=1.0,
                        )
                    else:
                        nc.vector.tensor_scalar(
                            out=zt[:, :gw],
                            in0=ps2[:],
                            scalar1=ecol[:, cb : cb + 1],
                            scalar2=None,
                            op0=addop,
                        )
                    nc.gpsimd.dma_start(
                        z_d[b, cb, :, j0 * FD : j1 * FD], zt[:, :gw]
                    )

        load_ph1(0)
        load_ph1(1)
        for b in range(BLOC):
            if b + 2 < BLOC:
                load_ph1(b + 2)
            ecol, w3 = schain(b)
            ph2_store(b, ecol, w3)

    nc.compile()
    return nc


def _get_nc():
    if "nc" not in _cache:
        _cache["nc"] = _build_nc()
    return _cache["nc"]


def kernel(x, g_w, g_b, theta_w, theta_b, phi_w, phi_b, W_w, W_b,
           bn_gamma, bn_beta, bn_mean, bn_var):
    import os

    from concourse.bass_utils import run_bass_kernel_spmd

    x = np.ascontiguousarray(np.asarray(x, dtype=np.float32))
    f = lambda a: np.asarray(a, dtype=np.float32).reshape(-1)
    g_w, g_b = f(g_w), f(g_b)
    theta_w, theta_b = f(theta_w), f(theta_b)
    phi_w, phi_b = f(phi_w), f(phi_b)
    W_w, W_b = f(W_w), f(W_b)
    bn_gamma, bn_beta = f(bn_gamma), f(bn_beta)
    bn_mean, bn_var = f(bn_mean), f(bn_var)

    inv_std = bn_gamma / np.sqrt(bn_var + BN_EPS)
    A = W_w * inv_std                       # (C,)
    D = (W_b - bn_mean) * inv_std + bn_beta  # (C,)

    w_u = g_w + phi_w
    w_v = g_w - phi_w
    wgp = np.empty((P, 4), np.float32)
    wgp[:, 0] = w_u[:P]
    wgp[:, 1] = w_v[:P]
    wgp[:, 2] = w_u[P:]
    wgp[:, 3] = w_v[P:]
    bgp = np.array([[g_b[0] + phi_b[0]], [g_b[0] - phi_b[0]]], np.float32)
    pm = np.array([[0.25 / N], [-0.25 / N]], np.float32)
    # w2b[k, kb*256 + c] = theta_w[kb*128 + k] * A[c]
    w2b = np.empty((P, 512), np.float32)
    w2b[:, 0:256] = theta_w[:P, None] * A[None, :]
    w2b[:, 256:512] = theta_w[P:, None] * A[None, :]
    i2 = np.zeros((P, 512), np.float32)
    i2[np.arange(P), np.arange(P)] = 1.0          # kb=0 block: rows k == cols c
    i2[np.arange(P), 256 + P + np.arange(P)] = 1.0  # kb=1: k+128 == c
    fd = np.empty((P, 4), np.float32)
    F = theta_b[0] * A
    fd[:, 0] = F[:P]
    fd[:, 1] = F[P:]
    fd[:, 2] = D[:P]
    fd[:, 3] = D[P:]

    consts = {
        "wgp": wgp, "bgp": bgp,
        "w2b": w2b, "i2": i2, "fd": fd, "pm": pm,
    }

    in_maps = []
    for i in range(NCORES):
        xs = np.ascontiguousarray(
            x[i * BLOC : (i + 1) * BLOC].reshape(BLOC, 2, P, N)
        )
        in_maps.append({"x": xs, **consts})

    nc = _get_nc()
    trace = os.environ.get("KERNEL_TRACE", "0") == "1"
    res = run_bass_kernel_spmd(
        nc, in_maps, core_ids=list(range(NCORES)), trace=trace
    )
    _cache["last_results"] = res

    z = np.concatenate(
        [res.results[i]["z"].reshape(BLOC, C, HH, WW) for i in range(NCORES)],
        axis=0,
    )
    return z
